# revision 17
# baseline (speedup 1.0000x reference)
"""nn_HR2O_NL on 8 Trainium2 NeuronCores.

Pipeline (matches the oracle):
  q,k,v = conv3x3(x, w_{q,k,v})   (SAME pad)
  att[i,j,h,w] = sum_c q[i,c,h,w] k[j,c,h,w] / sqrt(C); softmax over j
  virt = att @ v;  GroupNorm(1, C) + relu;  conv3x3(w_o);  out = x + virt

Sharding: H rows split 4-per-core (8 cores). Each core computes a 6-row
band of q/k/v and attention (own 4 rows + 1 halo row each side), so the
per-pixel N-by-N attention needs no collective. GroupNorm stats (sum,
sumsq per instance) are AllReduce'd (512 bytes).

Convs run as 9 shifted 1x1-conv matmuls in bf16 accumulating in PSUM.
W is padded to 34 with zero columns so the kernel-window shifts are pure
AP offsets. q/k are stored instance-minor [c, (y,x,i)] so per-pixel
attention operands are contiguous; v is DMA-transposed (XBAR) to
[(px,j), c] tiles for the att@v matmul.
"""

import math
from contextlib import ExitStack

import numpy as np
import ml_dtypes

import sys
if "/opt/trn_rl_repo" not in sys.path:
    sys.path.insert(0, "/opt/trn_rl_repo")

import concourse.bass as bass
import concourse.mybir as mybir
import concourse.tile as tile
from concourse import bacc

BF16 = mybir.dt.bfloat16
F32 = mybir.dt.float32
AF = mybir.ActivationFunctionType
ALU = mybir.AluOpType

N_CORES = 8
NI = 32          # instances
C = 512          # channels
CC = 4           # channel chunks of 128
H = W = 32
YB = 8           # x rows per core (4 own + 2 halo each side)
G = 6            # virt grid rows per core (4 own + 1 halo each side)
YO = 4           # own rows per core
WP = 34          # padded width
CHW = C * H * W
EPS = 1e-5

_CACHE = {}


# ---------------------------------------------------------------- program ---

def _build_program():
    nc = bacc.Bacc("TRN2", target_bir_lowering=False, debug=False,
                   num_devices=N_CORES)

    xb_h = nc.dram_tensor("xb", [CC, 128, NI * YB * WP], BF16, kind="ExternalInput")
    xr_h = nc.dram_tensor("xr", [CC, 128, NI * YO * W], F32, kind="ExternalInput")
    wqkv_h = nc.dram_tensor("wqkv", [12, 128, CC * 9 * 128], BF16, kind="ExternalInput")
    wo_h = nc.dram_tensor("wo", [CC, 128, CC * 9 * 128], BF16, kind="ExternalInput")
    gam_h = nc.dram_tensor("gam", [CC, 128, 1], F32, kind="ExternalInput")
    bet_h = nc.dram_tensor("bet", [CC, 128, 1], F32, kind="ExternalInput")
    msk_h = nc.dram_tensor("msk", [128, G * WP], BF16, kind="ExternalInput")
    out_h = nc.dram_tensor("out", [NI, C, YO, W], F32, kind="ExternalOutput")

    with tile.TileContext(nc) as tc, ExitStack() as es:
        _emit(nc, tc, es, xb_h, xr_h, wqkv_h, wo_h, gam_h, bet_h, msk_h, out_h)
    nc.compile()
    return nc


def _emit(nc, tc, es, xb_h, xr_h, wqkv_h, wo_h, gam_h, bet_h, msk_h, out_h):
    pool = lambda *a, **kw: es.enter_context(tc.tile_pool(*a, **kw))

    x_pool = pool(name="x", bufs=CC)
    w_pool = pool(name="w", bufs=2)
    qkv_pool = pool(name="qkv", bufs=12)
    vt_pool = pool(name="vt", bufs=8)
    att_pool = pool(name="att", bufs=3)
    virt_pool = pool(name="virt", bufs=CC)
    stat_pool = pool(name="stat", bufs=1)
    io_pool = pool(name="io", bufs=2)
    cpsum = pool(name="cpsum", bufs=3, space="PSUM")
    apsum = pool(name="apsum", bufs=2, space="PSUM")
    vpsum = pool(name="vpsum", bufs=2, space="PSUM")
    spsum = pool(name="spsum", bufs=1, space="PSUM")
    dram = pool(name="dram", bufs=2 * CC + 2, space="DRAM")

    # ---- load x, gamma/beta, mask, constants
    x_t = []
    for cc in range(CC):
        t = x_pool.tile([128, NI * YB * WP], BF16, tag="x")
        nc.sync.dma_start(t[:], xb_h.ap()[cc])
        x_t.append(t)
    gam_t, bet_t = [], []
    for cc in range(CC):
        g = stat_pool.tile([128, 1], F32, tag=f"gam{cc}")
        nc.sync.dma_start(g[:], gam_h.ap()[cc])
        gam_t.append(g)
        b = stat_pool.tile([128, 1], F32, tag=f"bet{cc}")
        nc.sync.dma_start(b[:], bet_h.ap()[cc])
        bet_t.append(b)
    msk_t = stat_pool.tile([128, G * WP], BF16, tag="msk")
    nc.sync.dma_start(msk_t[:], msk_h.ap()[:])
    ones_t = stat_pool.tile([128, 1], F32, tag="ones")
    nc.vector.memset(ones_t[:], 1.0)
    eps_t = stat_pool.tile([128, 1], F32, tag="eps")
    nc.vector.memset(eps_t[:], EPS)

    stats_ps = spsum.tile([1, 64], F32, tag="stats")

    virt_dram = [[dram.tile([128, NI * 3 * WP], BF16, tag=f"vd{yg}{cc}",
                            name=f"vd{yg}{cc}")
                  for cc in range(CC)] for yg in range(2)]

    n_stat_mm = 0

    # ================= phase 1: per ygroup conv qkv + attention ==============
    for yg in range(2):
        g0 = 3 * yg  # grid rows [g0, g0+3)

        # ---- conv q/k/v for grid rows g0..g0+2 (all 32 instances)
        qkv_t = [[None] * CC for _ in range(3)]  # [kind][cc]
        for kind in range(3):
            for cc in range(CC):
                qkv_t[kind][cc] = qkv_pool.tile([128, 3 * W * NI], BF16,
                                                tag="qkv", name=f"qkv{kind}{cc}")

        for oc in range(12):
            w_t = w_pool.tile([128, CC * 9 * 128], BF16, tag="w")
            nc.sync.dma_start(w_t[:], wqkv_h.ap()[oc])
            w_r = w_t[:].rearrange("p (cc s co) -> p cc s co", cc=CC, s=9)
            kind, occ = oc // CC, oc % CC
            dst_t = qkv_t[kind][occ]
            dst_r = dst_t[:].rearrange("p (y xc j) -> p y j xc", y=3, xc=W, j=NI)
            for gl in range(3):           # local grid row
                gg = g0 + gl              # grid row; x rows gg..gg+2
                ps = [cpsum.tile([128, 512], F32, tag="cps", name=f"cps{_ih}")
                      for _ih in range(2)]
                for cc in range(CC):
                    x_r = x_t[cc][:].rearrange("p (i y xc) -> p i y xc",
                                               i=NI, y=YB, xc=WP)
                    for s in range(9):
                        dy, dx = s // 3, s % 3
                        for ih in range(2):
                            rhs = x_r[:, ih * 16:(ih + 1) * 16, gg + dy,
                                      dx:dx + W]
                            nc.tensor.matmul(
                                ps[ih][:].rearrange("p (i xc) -> p i xc", i=16),
                                lhsT=w_r[:, cc, s, :], rhs=rhs,
                                start=(cc == 0 and s == 0),
                                stop=(cc == CC - 1 and s == 8))
                for ih in range(2):
                    nc.vector.tensor_copy(
                        dst_r[:, gl, ih * 16:(ih + 1) * 16, :],
                        ps[ih][:].rearrange("p (i xc) -> p i xc", i=16))

        # ---- attention for 96 pixels (3 rows x 32 cols), 4-px groups
        virt_t = []
        for cc in range(CC):
            t = virt_pool.tile([128, NI * 3 * WP], BF16, tag="virt")
            nc.vector.memset(t[:], 0.0)
            virt_t.append(t)

        for pg in range(24):
            y_loc, xc0 = pg // 8, (pg % 8) * 4
            # v [c, (4px, 32j)] -> vt4 [(4px, 32j), c] via XBAR, then rebase
            # each pixel's [32j, c] slab to partition 0 (PE can't read
            # operands at nonzero base partitions on this runtime)
            vt4 = []
            for cc in range(CC):
                vt = vt_pool.tile([128, 128], BF16, tag="vt", name=f"vt{cc}", bufs=6)
                nc.sync.dma_start(
                    vt[:], qkv_t[2][cc][:][:, pg * 128:(pg + 1) * 128],
                    transpose=True)
                vt4.append(vt)
            vtp = []
            for pxs in range(4):
                t = vt_pool.tile([32, 4 * 128], BF16, tag="vtp",
                                 name=f"vtp{pxs}", bufs=6)
                for cc in range(CC):
                    nc.sync.dma_start(
                        t[:][:, cc * 128:(cc + 1) * 128],
                        vt4[cc][pxs * 32:(pxs + 1) * 32, :])
                vtp.append(t)

            # att^T [j, (px, i)] directly: out partitions = j would need a
            # transposed q/k matmul; instead scores land [i, (px, j)]
            aps = apsum.tile([32, 128], F32, tag="aps")
            n = 0
            for pxs in range(4):
                px = pg * 4 + pxs
                for cc in range(CC):
                    nc.tensor.matmul(
                        aps[:, pxs * 32:(pxs + 1) * 32],
                        lhsT=qkv_t[0][cc][:][:, px * 32:(px + 1) * 32],
                        rhs=qkv_t[1][cc][:][:, px * 32:(px + 1) * 32],
                        start=(n == 0), stop=(n == 15))
                    n += 1

            exp_t = att_pool.tile([32, 128], BF16, tag="exp")
            nc.scalar.activation(exp_t[:], aps[:], AF.Exp,
                                 scale=1.0 / math.sqrt(C))
            exp_r = exp_t[:].rearrange("p (px j) -> p px j", px=4)
            rs_t = att_pool.tile([32, 4], F32, tag="rs")
            nc.vector.tensor_reduce(rs_t[:], exp_r,
                                    axis=mybir.AxisListType.X, op=ALU.add)
            rec_t = att_pool.tile([32, 4], F32, tag="rec")
            nc.vector.reciprocal(rec_t[:], rs_t[:])
            attn_t = att_pool.tile([32, 128], BF16, tag="attn")
            nc.vector.tensor_tensor(
                attn_t[:].rearrange("p (px j) -> p px j", px=4), exp_r,
                rec_t[:].broadcast_to([32, 4, 32]), op=ALU.mult)
            attT_t = att_pool.tile([32, 128], BF16, tag="attT")
            nc.vector.transpose(attT_t[:], attn_t[:])

            vps = vpsum.tile([128, 512], F32, tag="vps")
            n = 0
            for pxs in range(4):
                for cc in range(CC):
                    nc.tensor.matmul(
                        vps[:, (pxs * 4 + cc) * 32:(pxs * 4 + cc + 1) * 32],
                        lhsT=vtp[pxs][:][:, cc * 128:(cc + 1) * 128],
                        rhs=attT_t[:][:, pxs * 32:(pxs + 1) * 32],
                        start=(n == 0), stop=(n == 15))
                    n += 1
            vps_r = vps[:].rearrange("p (pxs cc i) -> p pxs cc i", pxs=4, cc=CC)
            for cc in range(CC):
                dst = virt_t[cc][:].rearrange("p (i y xc) -> p y xc i",
                                              i=NI, y=3, xc=WP)
                nc.vector.tensor_copy(
                    dst[:, y_loc, 1 + xc0:1 + xc0 + 4, :], vps_r[:, :, cc, :])

        # ---- GroupNorm partial stats over own rows of this ygroup
        oy = 1 - yg  # own grid rows local idx: yg0 -> rows 1,2 ; yg1 -> 0,1
        for cc in range(CC):
            v_r = virt_t[cc][:].rearrange("p (i y xc) -> p i y xc",
                                          i=NI, y=3, xc=WP)
            valid = v_r[:, :, oy:oy + 2, 1:1 + W]
            t1 = stat_pool.tile([128, 64], F32, tag="t1", bufs=2)
            nc.vector.tensor_reduce(
                t1[:].rearrange("p (i y) -> p i y", i=NI), valid,
                axis=mybir.AxisListType.X, op=ALU.add)
            s1 = stat_pool.tile([128, 32], F32, tag="s1", bufs=2)
            nc.vector.tensor_reduce(
                s1[:], t1[:].rearrange("p (i y) -> p i y", i=NI),
                axis=mybir.AxisListType.X, op=ALU.add)
            r2 = []
            for yr in range(2):
                sq = stat_pool.tile([128, NI * W], BF16, tag="sq", bufs=1)
                nc.scalar.square(
                    sq[:].rearrange("p (i xc) -> p i xc", i=NI),
                    v_r[:, :, oy + yr, 1:1 + W])
                r = stat_pool.tile([128, 32], F32, tag=f"r2{yr}", bufs=2)
                nc.vector.tensor_reduce(
                    r[:], sq[:].rearrange("p (i xc) -> p i xc", i=NI),
                    axis=mybir.AxisListType.X, op=ALU.add)
                r2.append(r)
            s2 = stat_pool.tile([128, 32], F32, tag="s2", bufs=2)
            nc.vector.tensor_add(s2[:], r2[0][:], r2[1][:])
            nc.tensor.matmul(stats_ps[:, 0:32], lhsT=ones_t[:], rhs=s1[:],
                             start=(n_stat_mm == 0),
                             stop=(yg == 1 and cc == CC - 1),
                             skip_group_check=True)
            nc.tensor.matmul(stats_ps[:, 32:64], lhsT=ones_t[:], rhs=s2[:],
                             start=False,
                             stop=(yg == 1 and cc == CC - 1),
                             skip_group_check=True)
            n_stat_mm += 1

        # ---- spill virt to DRAM
        for cc in range(CC):
            nc.sync.dma_start(virt_dram[yg][cc][:], virt_t[cc][:])

    # ================= phase 2: stats AllReduce + finalize ===================
    st_sb = stat_pool.tile([1, 64], F32, tag="stsb")
    nc.vector.tensor_copy(st_sb[:], stats_ps[:])
    bnc_in = dram.tile([1, 64], F32, tag="bin")
    bnc_out = dram.tile([1, 64], F32, tag="bout")
    nc.sync.dma_start(bnc_in[:], st_sb[:])
    nc.gpsimd.collective_compute(
        "AllReduce", ALU.add,
        replica_groups=[list(range(N_CORES))],
        ins=[bnc_in[:].opt()], outs=[bnc_out[:].opt()])

    # broadcast summed stats to all 128 partitions, finalize redundantly
    finb = stat_pool.tile([128, 64], F32, tag="finb")
    nc.sync.dma_start(finb[:], bnc_out[:].partition_broadcast(128).squeeze(1))
    mneg = stat_pool.tile([128, 32], F32, tag="mneg")
    ex2 = stat_pool.tile([128, 32], F32, tag="ex2")
    msq = stat_pool.tile([128, 32], F32, tag="msq")
    rstd = stat_pool.tile([128, 32], F32, tag="rstd")
    nc.scalar.mul(mneg[:], finb[:][:, 0:32], -1.0 / CHW)
    nc.scalar.mul(ex2[:], finb[:][:, 32:64], 1.0 / CHW)
    nc.scalar.square(msq[:], mneg[:])
    nc.vector.tensor_sub(ex2[:], ex2[:], msq[:])              # var
    nc.scalar.activation(msq[:], ex2[:], AF.Sqrt, bias=eps_t[:])  # sqrt(var+eps)
    nc.vector.reciprocal(rstd[:], msq[:])

    scale_t, bias_t = [], []
    for cc in range(CC):
        sc = stat_pool.tile([128, 32], F32, tag=f"scale{cc}")
        nc.vector.tensor_scalar(sc[:], rstd[:], gam_t[cc][:], None,
                                op0=ALU.mult)
        tb = stat_pool.tile([128, 32], F32, tag=f"tbias{cc}")
        nc.vector.tensor_mul(tb[:], mneg[:], sc[:])
        nc.vector.tensor_scalar(tb[:], tb[:], bet_t[cc][:], None, op0=ALU.add)
        scale_t.append(sc)
        bias_t.append(tb)

    # ================= phase 3: norm + relu + mask, conv_o + residual =======
    nrm_t = []
    for cc in range(CC):
        nt = x_pool.tile([128, G * NI * WP], BF16, tag="x", name=f"nrm{cc}")
        nrm_t.append(nt)
        nt_g = nt[:].rearrange("p (g i xc) -> p g i xc", g=G, i=NI)
        for g in range(G):
            yg, gl = g // 3, g % 3
            src = virt_dram[yg][cc][:].rearrange("p (i y xc) -> p i y xc",
                                                 i=NI, y=3, xc=WP)
            nc.sync.dma_start(nt_g[:, g, :, :], src[:, :, gl, :])
            nt_r = nt_g[:, g, :, :]
            sc_b = scale_t[cc][:].broadcast_to([128, 32, WP])
            bi_b = bias_t[cc][:].broadcast_to([128, 32, WP])
            nc.vector.tensor_tensor(nt_r, nt_r, sc_b, op=ALU.mult)
            nc.vector.tensor_tensor(nt_r, nt_r, bi_b, op=ALU.add)
            nc.vector.tensor_scalar_max(nt_r, nt_r, 0.0)
            m_b = (msk_t[:][:, g * WP:(g + 1) * WP]
                   .broadcast_to([128, WP, 32]).transpose([0, 2, 1]))
            nc.vector.tensor_tensor(nt_r, nt_r, m_b, op=ALU.mult)

    for oc in range(CC):
        w_t = w_pool.tile([128, CC * 9 * 128], BF16, tag="w")
        nc.sync.dma_start(w_t[:], wo_h.ap()[oc])
        w_r = w_t[:].rearrange("p (cc s co) -> p cc s co", cc=CC, s=9)
        for yo in range(YO):            # out grid row = yo + 1
            ps = [cpsum.tile([128, 512], F32, tag="cps", name=f"cps{_ih}")
                      for _ih in range(2)]
            for cc in range(CC):
                for s in range(9):
                    dy, dx = s // 3, s % 3
                    g = yo + dy          # virt grid row
                    rhs_full = nrm_t[cc][:].rearrange(
                        "p (g i xc) -> p g i xc", g=G, i=NI)
                    for ih in range(2):
                        rhs = rhs_full[:, g, ih * 16:(ih + 1) * 16, dx:dx + W]
                        nc.tensor.matmul(
                            ps[ih][:].rearrange("p (i xc) -> p i xc", i=16),
                            lhsT=w_r[:, cc, s, :], rhs=rhs,
                            start=(cc == 0 and s == 0),
                            stop=(cc == CC - 1 and s == 8))
            xr_r = xr_h.ap()[oc].rearrange("p (i y xc) -> p i y xc",
                                           i=NI, y=YO, xc=W)
            for ih in range(2):
                xres = io_pool.tile([128, 512], F32, tag="xres")
                nc.sync.dma_start(
                    xres[:].rearrange("p (i xc) -> p i xc", i=16),
                    xr_r[:, ih * 16:(ih + 1) * 16, yo, :])
                out_t = io_pool.tile([128, 512], F32, tag="outt")
                nc.vector.tensor_add(out_t[:], ps[ih][:], xres[:])
                dst = (out_h.ap()[ih * 16:(ih + 1) * 16,
                                  oc * 128:(oc + 1) * 128, yo, :]
                       .transpose([1, 0, 2]))
                nc.sync.dma_start(
                    dst, out_t[:].rearrange("p (i xc) -> p i xc", i=16))


# ---------------------------------------------------------------- host prep --

def _prep_inputs(x, w_q, w_k, w_v, w_o, gamma, beta):
    x = np.asarray(x, np.float32)
    bf = ml_dtypes.bfloat16

    # x padded: rows +2 shift, cols +1 shift
    xp = np.zeros((NI, C, H + 4, WP), np.float32)
    xp[:, :, 2:2 + H, 1:1 + W] = x

    wqkv = np.concatenate([np.asarray(w, np.float32).reshape(C, C, 9)
                           for w in (w_q, w_k, w_v)], axis=0)  # [1536, C, 9]
    # [oc, ci, (cc, s, co)]
    wq = wqkv.reshape(12, 128, CC, 128, 9).transpose(0, 3, 2, 4, 1)
    wq = np.ascontiguousarray(wq.reshape(12, 128, CC * 9 * 128)).astype(bf)
    wo = np.asarray(w_o, np.float32).reshape(CC, 128, CC, 128, 9)
    wo = wo.transpose(0, 3, 2, 4, 1)
    wo = np.ascontiguousarray(wo.reshape(CC, 128, CC * 9 * 128)).astype(bf)

    gam = np.ascontiguousarray(np.asarray(gamma, np.float32)
                               .reshape(CC, 128, 1))
    bet = np.ascontiguousarray(np.asarray(beta, np.float32)
                               .reshape(CC, 128, 1))

    in_maps = []
    for r in range(N_CORES):
        y0 = 4 * r
        xs = xp[:, :, y0:y0 + YB, :]                       # [NI, C, 8, 34]
        xb = xs.reshape(NI, CC, 128, YB, WP).transpose(1, 2, 0, 3, 4)
        xb = np.ascontiguousarray(xb.reshape(CC, 128, NI * YB * WP)).astype(bf)
        xr = x[:, :, y0:y0 + YO, :].reshape(NI, CC, 128, YO, W)
        xr = xr.transpose(1, 2, 0, 3, 4)
        xr = np.ascontiguousarray(xr.reshape(CC, 128, NI * YO * W))

        msk = np.zeros((G, WP), np.float32)
        msk[:, 1:1 + W] = 1.0
        if r == 0:
            msk[0] = 0.0
        if r == N_CORES - 1:
            msk[G - 1] = 0.0
        msk = np.repeat(msk.reshape(1, G * WP), 128, axis=0).astype(bf)

        in_maps.append({"xb": xb, "xr": xr, "wqkv": wq, "wo": wo,
                        "gam": gam, "bet": bet, "msk": msk})
    return in_maps


def _gather(results):
    out = np.empty((NI, C, H, W), np.float32)
    for r in range(N_CORES):
        out[:, :, 4 * r:4 * r + YO, :] = results[r]["out"]
    return out


def _get_nc():
    if "nc" not in _CACHE:
        _CACHE["nc"] = _build_program()
    return _CACHE["nc"]


def _run(in_maps, trace=False):
    from concourse.bass_utils import run_bass_kernel_spmd
    nc = _get_nc()
    return run_bass_kernel_spmd(nc, in_maps, list(range(N_CORES)), trace=trace)


def kernel(x, w_q, w_k, w_v, w_o, gamma, beta):
    in_maps = _prep_inputs(x, w_q, w_k, w_v, w_o, gamma, beta)
    res = _run(in_maps, trace=False)
    return _gather(res.results)


def kernel_profiled(x, w_q, w_k, w_v, w_o, gamma, beta):
    """Returns (output, exec_time_ns or None)."""
    in_maps = _prep_inputs(x, w_q, w_k, w_v, w_o, gamma, beta)
    res = _run(in_maps, trace=False)
    return _gather(res.results), None


# revision 19
# speedup vs baseline: 185.5488x; 185.5488x over previous
"""nn_HR2O_NL on 8 Trainium2 NeuronCores.

Pipeline (matches the oracle):
  q,k,v = conv3x3(x, w_{q,k,v})   (SAME pad)
  att[i,j,h,w] = sum_c q[i,c,h,w] k[j,c,h,w] / sqrt(C); softmax over j
  virt = att @ v;  GroupNorm(1, C) + relu;  conv3x3(w_o);  out = x + virt

Sharding: H rows split 4-per-core (8 cores). Each core computes a 6-row
band of q/k/v and attention (own 4 rows + 1 halo row each side), so the
per-pixel N-by-N attention needs no collective. GroupNorm stats (sum,
sumsq per instance) are AllReduce'd (512 bytes).

Convs run as 9 shifted 1x1-conv matmuls in bf16 accumulating in PSUM.
W is padded to 34 with zero columns so the kernel-window shifts are pure
AP offsets. q/k are stored instance-minor [c, (y,x,i)] so per-pixel
attention operands are contiguous; v is DMA-transposed (XBAR) to
[(px,j), c] tiles for the att@v matmul.
"""

import math
from contextlib import ExitStack

import numpy as np
import ml_dtypes

import sys
if "/opt/trn_rl_repo" not in sys.path:
    sys.path.insert(0, "/opt/trn_rl_repo")

import concourse.bass as bass
import concourse.mybir as mybir
import concourse.tile as tile
from concourse import bacc

BF16 = mybir.dt.bfloat16
F32 = mybir.dt.float32
AF = mybir.ActivationFunctionType
ALU = mybir.AluOpType

N_CORES = 8
NI = 32          # instances
C = 512          # channels
CC = 4           # channel chunks of 128
H = W = 32
YB = 8           # x rows per core (4 own + 2 halo each side)
G = 6            # virt grid rows per core (4 own + 1 halo each side)
YO = 4           # own rows per core
WP = 34          # padded width
CHW = C * H * W
EPS = 1e-5

_CACHE = {}


# ---------------------------------------------------------------- program ---

def _build_program():
    nc = bacc.Bacc("TRN2", target_bir_lowering=False, debug=False,
                   num_devices=N_CORES)

    xb_h = nc.dram_tensor("xb", [CC, 128, NI * YB * WP], BF16, kind="ExternalInput")
    xr_h = nc.dram_tensor("xr", [CC, 128, NI * YO * W], F32, kind="ExternalInput")
    wqkv_h = nc.dram_tensor("wqkv", [12, 128, CC * 9 * 128], BF16, kind="ExternalInput")
    wo_h = nc.dram_tensor("wo", [CC, 128, CC * 9 * 128], BF16, kind="ExternalInput")
    gam_h = nc.dram_tensor("gam", [CC, 128, 1], F32, kind="ExternalInput")
    bet_h = nc.dram_tensor("bet", [CC, 128, 1], F32, kind="ExternalInput")
    msk_h = nc.dram_tensor("msk", [128, G * WP], BF16, kind="ExternalInput")
    out_h = nc.dram_tensor("out", [NI, C, YO, W], F32, kind="ExternalOutput")

    with tile.TileContext(nc) as tc, ExitStack() as es:
        _emit(nc, tc, es, xb_h, xr_h, wqkv_h, wo_h, gam_h, bet_h, msk_h, out_h)
    nc.compile()
    return nc


def _emit(nc, tc, es, xb_h, xr_h, wqkv_h, wo_h, gam_h, bet_h, msk_h, out_h):
    pool = lambda *a, **kw: es.enter_context(tc.tile_pool(*a, **kw))

    x_pool = pool(name="x", bufs=CC)
    w_pool = pool(name="w", bufs=2)
    qkv_pool = pool(name="qkv", bufs=12)
    vt_pool = pool(name="vt", bufs=8)
    att_pool = pool(name="att", bufs=3)
    virt_pool = pool(name="virt", bufs=CC)
    stat_pool = pool(name="stat", bufs=1)
    io_pool = pool(name="io", bufs=2)
    cpsum = pool(name="cpsum", bufs=3, space="PSUM")
    apsum = pool(name="apsum", bufs=2, space="PSUM")
    vpsum = pool(name="vpsum", bufs=2, space="PSUM")
    spsum = pool(name="spsum", bufs=1, space="PSUM")
    dram = pool(name="dram", bufs=2 * CC + 2, space="DRAM")

    # ---- load x, gamma/beta, mask, constants
    x_t = []
    for cc in range(CC):
        t = x_pool.tile([128, NI * YB * WP], BF16, tag="x")
        nc.sync.dma_start(t[:], xb_h.ap()[cc])
        x_t.append(t)
    gam_t, bet_t = [], []
    for cc in range(CC):
        g = stat_pool.tile([128, 1], F32, tag=f"gam{cc}")
        nc.sync.dma_start(g[:], gam_h.ap()[cc])
        gam_t.append(g)
        b = stat_pool.tile([128, 1], F32, tag=f"bet{cc}")
        nc.sync.dma_start(b[:], bet_h.ap()[cc])
        bet_t.append(b)
    msk_t = stat_pool.tile([128, G * WP], BF16, tag="msk")
    nc.sync.dma_start(msk_t[:], msk_h.ap()[:])
    ones_t = stat_pool.tile([128, 1], F32, tag="ones")
    nc.vector.memset(ones_t[:], 1.0)
    eps_t = stat_pool.tile([128, 1], F32, tag="eps")
    nc.vector.memset(eps_t[:], EPS)

    stats_ps = spsum.tile([1, 64], F32, tag="stats")

    virt_dram = [[dram.tile([128, NI * 3 * WP], BF16, tag=f"vd{yg}{cc}",
                            name=f"vd{yg}{cc}")
                  for cc in range(CC)] for yg in range(2)]

    n_stat_mm = 0

    # ================= phase 1: per ygroup conv qkv + attention ==============
    for yg in range(2):
        g0 = 3 * yg  # grid rows [g0, g0+3)

        # ---- conv q/k/v for grid rows g0..g0+2 (all 32 instances)
        qkv_t = [[None] * CC for _ in range(3)]  # [kind][cc]
        for kind in range(3):
            for cc in range(CC):
                qkv_t[kind][cc] = qkv_pool.tile([128, 3 * W * NI], BF16,
                                                tag="qkv", name=f"qkv{kind}{cc}")

        for oc in range(12):
            w_t = w_pool.tile([128, CC * 9 * 128], BF16, tag="w")
            nc.sync.dma_start(w_t[:], wqkv_h.ap()[oc])
            w_r = w_t[:].rearrange("p (cc s co) -> p cc s co", cc=CC, s=9)
            kind, occ = oc // CC, oc % CC
            dst_t = qkv_t[kind][occ]
            dst_r = dst_t[:].rearrange("p (y xc j) -> p y j xc", y=3, xc=W, j=NI)
            for gl in range(3):           # local grid row
                gg = g0 + gl              # grid row; x rows gg..gg+2
                ps = [cpsum.tile([128, 512], F32, tag="cps", name=f"cps{_ih}")
                      for _ih in range(2)]
                for cc in range(CC):
                    x_r = x_t[cc][:].rearrange("p (i y xc) -> p i y xc",
                                               i=NI, y=YB, xc=WP)
                    for s in range(9):
                        dy, dx = s // 3, s % 3
                        for ih in range(2):
                            rhs = x_r[:, ih * 16:(ih + 1) * 16, gg + dy,
                                      dx:dx + W]
                            nc.tensor.matmul(
                                ps[ih][:].rearrange("p (i xc) -> p i xc", i=16),
                                lhsT=w_r[:, cc, s, :], rhs=rhs,
                                start=(cc == 0 and s == 0),
                                stop=(cc == CC - 1 and s == 8))
                for ih in range(2):
                    nc.vector.tensor_copy(
                        dst_r[:, gl, ih * 16:(ih + 1) * 16, :],
                        ps[ih][:].rearrange("p (i xc) -> p i xc", i=16))

        # ---- attention for 96 pixels (3 rows x 32 cols), 4-px groups
        virt_t = []
        for cc in range(CC):
            t = virt_pool.tile([128, NI * 3 * WP], BF16, tag="virt")
            nc.vector.memset(t[:], 0.0)
            virt_t.append(t)

        for pg in range(24):
            y_loc, xc0 = pg // 8, (pg % 8) * 4
            # v [c, (4px, 32j)] -> vt4 [(4px, 32j), c] via XBAR, then rebase
            # each pixel's [32j, c] slab to partition 0 (PE can't read
            # operands at nonzero base partitions on this runtime)
            vt4 = []
            for cc in range(CC):
                vt = vt_pool.tile([128, 128], BF16, tag="vt", name=f"vt{cc}", bufs=6)
                nc.sync.dma_start(
                    vt[:], qkv_t[2][cc][:][:, pg * 128:(pg + 1) * 128],
                    transpose=True)
                vt4.append(vt)
            vtp = []
            for pxs in range(4):
                t = vt_pool.tile([32, 4 * 128], BF16, tag="vtp",
                                 name=f"vtp{pxs}", bufs=6)
                for cc in range(CC):
                    nc.sync.dma_start(
                        t[:][:, cc * 128:(cc + 1) * 128],
                        vt4[cc][pxs * 32:(pxs + 1) * 32, :])
                vtp.append(t)

            # att^T [j, (px, i)] directly: out partitions = j would need a
            # transposed q/k matmul; instead scores land [i, (px, j)]
            aps = apsum.tile([32, 128], F32, tag="aps")
            n = 0
            for pxs in range(4):
                px = pg * 4 + pxs
                for cc in range(CC):
                    nc.tensor.matmul(
                        aps[:, pxs * 32:(pxs + 1) * 32],
                        lhsT=qkv_t[0][cc][:][:, px * 32:(px + 1) * 32],
                        rhs=qkv_t[1][cc][:][:, px * 32:(px + 1) * 32],
                        start=(n == 0), stop=(n == 15))
                    n += 1

            exp_t = att_pool.tile([32, 128], BF16, tag="exp")
            nc.scalar.activation(exp_t[:], aps[:], AF.Exp,
                                 scale=1.0 / math.sqrt(C))
            exp_r = exp_t[:].rearrange("p (px j) -> p px j", px=4)
            rs_t = att_pool.tile([32, 4], F32, tag="rs")
            nc.vector.tensor_reduce(rs_t[:], exp_r,
                                    axis=mybir.AxisListType.X, op=ALU.add)
            rec_t = att_pool.tile([32, 4], F32, tag="rec")
            nc.vector.reciprocal(rec_t[:], rs_t[:])
            attn_t = att_pool.tile([32, 128], BF16, tag="attn")
            nc.vector.tensor_tensor(
                attn_t[:].rearrange("p (px j) -> p px j", px=4), exp_r,
                rec_t[:].broadcast_to([32, 4, 32]), op=ALU.mult)
            attT_t = att_pool.tile([32, 128], BF16, tag="attT")
            nc.vector.transpose(attT_t[:], attn_t[:])

            vps = vpsum.tile([128, 512], F32, tag="vps")
            n = 0
            for pxs in range(4):
                for cc in range(CC):
                    nc.tensor.matmul(
                        vps[:, (pxs * 4 + cc) * 32:(pxs * 4 + cc + 1) * 32],
                        lhsT=vtp[pxs][:][:, cc * 128:(cc + 1) * 128],
                        rhs=attT_t[:][:, pxs * 32:(pxs + 1) * 32],
                        start=(n == 0), stop=(n == 15))
                    n += 1
            vps_r = vps[:].rearrange("p (pxs cc i) -> p pxs cc i", pxs=4, cc=CC)
            for cc in range(CC):
                dst = virt_t[cc][:].rearrange("p (i y xc) -> p y xc i",
                                              i=NI, y=3, xc=WP)
                nc.vector.tensor_copy(
                    dst[:, y_loc, 1 + xc0:1 + xc0 + 4, :], vps_r[:, :, cc, :])

        # ---- GroupNorm partial stats over own rows of this ygroup
        oy = 1 - yg  # own grid rows local idx: yg0 -> rows 1,2 ; yg1 -> 0,1
        for cc in range(CC):
            v_r = virt_t[cc][:].rearrange("p (i y xc) -> p i y xc",
                                          i=NI, y=3, xc=WP)
            valid = v_r[:, :, oy:oy + 2, 1:1 + W]
            t1 = stat_pool.tile([128, 64], F32, tag="t1", bufs=2)
            nc.vector.tensor_reduce(
                t1[:].rearrange("p (i y) -> p i y", i=NI), valid,
                axis=mybir.AxisListType.X, op=ALU.add)
            s1 = stat_pool.tile([128, 32], F32, tag="s1", bufs=2)
            nc.vector.tensor_reduce(
                s1[:], t1[:].rearrange("p (i y) -> p i y", i=NI),
                axis=mybir.AxisListType.X, op=ALU.add)
            r2 = []
            for yr in range(2):
                sq = stat_pool.tile([128, NI * W], BF16, tag="sq", bufs=1)
                nc.scalar.square(
                    sq[:].rearrange("p (i xc) -> p i xc", i=NI),
                    v_r[:, :, oy + yr, 1:1 + W])
                r = stat_pool.tile([128, 32], F32, tag=f"r2{yr}", bufs=2)
                nc.vector.tensor_reduce(
                    r[:], sq[:].rearrange("p (i xc) -> p i xc", i=NI),
                    axis=mybir.AxisListType.X, op=ALU.add)
                r2.append(r)
            s2 = stat_pool.tile([128, 32], F32, tag="s2", bufs=2)
            nc.vector.tensor_add(s2[:], r2[0][:], r2[1][:])
            nc.tensor.matmul(stats_ps[:, 0:32], lhsT=ones_t[:], rhs=s1[:],
                             start=(n_stat_mm == 0),
                             stop=(yg == 1 and cc == CC - 1),
                             skip_group_check=True)
            nc.tensor.matmul(stats_ps[:, 32:64], lhsT=ones_t[:], rhs=s2[:],
                             start=False,
                             stop=(yg == 1 and cc == CC - 1),
                             skip_group_check=True)
            n_stat_mm += 1

        # ---- spill virt to DRAM
        for cc in range(CC):
            nc.sync.dma_start(virt_dram[yg][cc][:], virt_t[cc][:])

    # ================= phase 2: stats AllReduce + finalize ===================
    st_sb = stat_pool.tile([1, 64], F32, tag="stsb")
    nc.vector.tensor_copy(st_sb[:], stats_ps[:])
    bnc_in = dram.tile([1, 64], F32, tag="bin")
    bnc_out = dram.tile([1, 64], F32, tag="bout")
    nc.sync.dma_start(bnc_in[:], st_sb[:])
    nc.gpsimd.collective_compute(
        "AllReduce", ALU.add,
        replica_groups=[list(range(N_CORES))],
        ins=[bnc_in[:].opt()], outs=[bnc_out[:].opt()])

    # broadcast summed stats to all 128 partitions, finalize redundantly
    finb = stat_pool.tile([128, 64], F32, tag="finb")
    nc.sync.dma_start(finb[:], bnc_out[:].partition_broadcast(128).squeeze(1))
    mneg = stat_pool.tile([128, 32], F32, tag="mneg")
    ex2 = stat_pool.tile([128, 32], F32, tag="ex2")
    msq = stat_pool.tile([128, 32], F32, tag="msq")
    rstd = stat_pool.tile([128, 32], F32, tag="rstd")
    nc.scalar.mul(mneg[:], finb[:][:, 0:32], -1.0 / CHW)
    nc.scalar.mul(ex2[:], finb[:][:, 32:64], 1.0 / CHW)
    nc.scalar.square(msq[:], mneg[:])
    nc.vector.tensor_sub(ex2[:], ex2[:], msq[:])              # var
    nc.scalar.activation(msq[:], ex2[:], AF.Sqrt, bias=eps_t[:])  # sqrt(var+eps)
    nc.vector.reciprocal(rstd[:], msq[:])

    scale_t, bias_t = [], []
    for cc in range(CC):
        sc = stat_pool.tile([128, 32], F32, tag=f"scale{cc}")
        nc.vector.tensor_scalar(sc[:], rstd[:], gam_t[cc][:], None,
                                op0=ALU.mult)
        tb = stat_pool.tile([128, 32], F32, tag=f"tbias{cc}")
        nc.vector.tensor_mul(tb[:], mneg[:], sc[:])
        nc.vector.tensor_scalar(tb[:], tb[:], bet_t[cc][:], None, op0=ALU.add)
        scale_t.append(sc)
        bias_t.append(tb)

    # ================= phase 3: norm + relu + mask, conv_o + residual =======
    nrm_t = []
    for cc in range(CC):
        nt = x_pool.tile([128, G * NI * WP], BF16, tag="x", name=f"nrm{cc}")
        nrm_t.append(nt)
        nt_g = nt[:].rearrange("p (g i xc) -> p g i xc", g=G, i=NI)
        for g in range(G):
            yg, gl = g // 3, g % 3
            src = virt_dram[yg][cc][:].rearrange("p (i y xc) -> p i y xc",
                                                 i=NI, y=3, xc=WP)
            nc.sync.dma_start(nt_g[:, g, :, :], src[:, :, gl, :])
            nt_r = nt_g[:, g, :, :]
            sc_b = scale_t[cc][:].broadcast_to([128, 32, WP])
            bi_b = bias_t[cc][:].broadcast_to([128, 32, WP])
            nc.vector.tensor_tensor(nt_r, nt_r, sc_b, op=ALU.mult)
            nc.vector.tensor_tensor(nt_r, nt_r, bi_b, op=ALU.add)
            nc.vector.tensor_scalar_max(nt_r, nt_r, 0.0)
            m_b = (msk_t[:][:, g * WP:(g + 1) * WP]
                   .broadcast_to([128, WP, 32]).transpose([0, 2, 1]))
            nc.vector.tensor_tensor(nt_r, nt_r, m_b, op=ALU.mult)

    for oc in range(CC):
        w_t = w_pool.tile([128, CC * 9 * 128], BF16, tag="w")
        nc.sync.dma_start(w_t[:], wo_h.ap()[oc])
        w_r = w_t[:].rearrange("p (cc s co) -> p cc s co", cc=CC, s=9)
        for yo in range(YO):            # out grid row = yo + 1
            ps = [cpsum.tile([128, 512], F32, tag="cps", name=f"cps{_ih}")
                      for _ih in range(2)]
            for cc in range(CC):
                for s in range(9):
                    dy, dx = s // 3, s % 3
                    g = yo + dy          # virt grid row
                    rhs_full = nrm_t[cc][:].rearrange(
                        "p (g i xc) -> p g i xc", g=G, i=NI)
                    for ih in range(2):
                        rhs = rhs_full[:, g, ih * 16:(ih + 1) * 16, dx:dx + W]
                        nc.tensor.matmul(
                            ps[ih][:].rearrange("p (i xc) -> p i xc", i=16),
                            lhsT=w_r[:, cc, s, :], rhs=rhs,
                            start=(cc == 0 and s == 0),
                            stop=(cc == CC - 1 and s == 8))
            xr_r = xr_h.ap()[oc].rearrange("p (i y xc) -> p i y xc",
                                           i=NI, y=YO, xc=W)
            for ih in range(2):
                xres = io_pool.tile([128, 512], F32, tag="xres")
                nc.sync.dma_start(
                    xres[:].rearrange("p (i xc) -> p i xc", i=16),
                    xr_r[:, ih * 16:(ih + 1) * 16, yo, :])
                out_t = io_pool.tile([128, 512], F32, tag="outt")
                nc.vector.tensor_add(out_t[:], ps[ih][:], xres[:])
                dst = (out_h.ap()[ih * 16:(ih + 1) * 16,
                                  oc * 128:(oc + 1) * 128, yo, :]
                       .transpose([1, 0, 2]))
                nc.sync.dma_start(
                    dst, out_t[:].rearrange("p (i xc) -> p i xc", i=16))


# ---------------------------------------------------------------- host prep --

def _prep_inputs(x, w_q, w_k, w_v, w_o, gamma, beta):
    x = np.asarray(x, np.float32)
    bf = ml_dtypes.bfloat16

    # x padded: rows +2 shift, cols +1 shift
    xp = np.zeros((NI, C, H + 4, WP), np.float32)
    xp[:, :, 2:2 + H, 1:1 + W] = x

    wqkv = np.concatenate([np.asarray(w, np.float32).reshape(C, C, 9)
                           for w in (w_q, w_k, w_v)], axis=0)  # [1536, C, 9]
    # [oc, ci, (cc, s, co)]
    wq = wqkv.reshape(12, 128, CC, 128, 9).transpose(0, 3, 2, 4, 1)
    wq = np.ascontiguousarray(wq.reshape(12, 128, CC * 9 * 128)).astype(bf)
    wo = np.asarray(w_o, np.float32).reshape(CC, 128, CC, 128, 9)
    wo = wo.transpose(0, 3, 2, 4, 1)
    wo = np.ascontiguousarray(wo.reshape(CC, 128, CC * 9 * 128)).astype(bf)

    gam = np.ascontiguousarray(np.asarray(gamma, np.float32)
                               .reshape(CC, 128, 1))
    bet = np.ascontiguousarray(np.asarray(beta, np.float32)
                               .reshape(CC, 128, 1))

    in_maps = []
    for r in range(N_CORES):
        y0 = 4 * r
        xs = xp[:, :, y0:y0 + YB, :]                       # [NI, C, 8, 34]
        xb = xs.reshape(NI, CC, 128, YB, WP).transpose(1, 2, 0, 3, 4)
        xb = np.ascontiguousarray(xb.reshape(CC, 128, NI * YB * WP)).astype(bf)
        xr = x[:, :, y0:y0 + YO, :].reshape(NI, CC, 128, YO, W)
        xr = xr.transpose(1, 2, 0, 3, 4)
        xr = np.ascontiguousarray(xr.reshape(CC, 128, NI * YO * W))

        msk = np.zeros((G, WP), np.float32)
        msk[:, 1:1 + W] = 1.0
        if r == 0:
            msk[0] = 0.0
        if r == N_CORES - 1:
            msk[G - 1] = 0.0
        msk = np.repeat(msk.reshape(1, G * WP), 128, axis=0).astype(bf)

        in_maps.append({"xb": xb, "xr": xr, "wqkv": wq, "wo": wo,
                        "gam": gam, "bet": bet, "msk": msk})
    return in_maps


def _gather(results):
    out = np.empty((NI, C, H, W), np.float32)
    for r in range(N_CORES):
        out[:, :, 4 * r:4 * r + YO, :] = results[r]["out"]
    return out


def _get_nc():
    if "nc" not in _CACHE:
        _CACHE["nc"] = _build_program()
    return _CACHE["nc"]


def _get_exec():
    """Jitted 8-core shard_map executable + metadata, built once."""
    if "exec" in _CACHE:
        return _CACHE["exec"]
    import jax
    from jax.experimental.shard_map import shard_map
    from jax.sharding import Mesh, PartitionSpec
    from concourse import bass2jax, mybir as mb

    nc = _get_nc()
    bass2jax.install_neuronx_cc_hook()
    part_name = (nc.partition_id_tensor.name
                 if nc.partition_id_tensor else None)
    in_names, out_names, out_avals, zero_outs = [], [], [], []
    for alloc in nc.m.functions[0].allocations:
        if not isinstance(alloc, mb.MemoryLocationSet):
            continue
        name = alloc.memorylocations[0].name
        if alloc.kind == "ExternalInput":
            if name != part_name:
                in_names.append(name)
        elif alloc.kind == "ExternalOutput":
            shape = tuple(alloc.tensor_shape)
            dtype = mb.dt.np(alloc.dtype)
            out_names.append(name)
            out_avals.append(jax.core.ShapedArray(shape, dtype))
            zero_outs.append(np.zeros(shape, dtype))
    n_params = len(in_names)
    all_names = list(in_names + out_names)
    if part_name is not None:
        all_names.append(part_name)
    all_names = tuple(all_names)

    def _body(*args):
        operands = list(args)
        if part_name is not None:
            operands.append(bass2jax.partition_id_tensor())
        return tuple(bass2jax._bass_exec_p.bind(
            *operands,
            out_avals=tuple(out_avals), in_names=all_names,
            out_names=tuple(out_names), lowering_input_output_aliases=(),
            sim_require_finite=True, sim_require_nnan=True, nc=nc))

    devices = jax.devices()[:N_CORES]
    mesh = Mesh(np.asarray(devices), ("core",))
    n_outs = len(out_names)
    donate = tuple(range(n_params, n_params + n_outs))
    sharded = jax.jit(
        shard_map(_body, mesh=mesh,
                  in_specs=(PartitionSpec("core"),) * (n_params + n_outs),
                  out_specs=(PartitionSpec("core"),) * n_outs,
                  check_rep=False),
        donate_argnums=donate, keep_unused=True)
    _CACHE["exec"] = (sharded, in_names, out_names, out_avals, zero_outs,
                      mesh)
    return _CACHE["exec"]


def _run_fast(in_maps, time_iters=0):
    """Execute on 8 cores; optionally re-run to measure per-call time."""
    import time as _time
    import jax
    from jax.sharding import NamedSharding, PartitionSpec

    sharded, in_names, out_names, out_avals, zero_outs, mesh = _get_exec()
    sh = NamedSharding(mesh, PartitionSpec("core"))
    concat_in = [
        jax.device_put(
            np.concatenate([np.asarray(m[name]) for m in in_maps], axis=0), sh)
        for name in in_names]
    def zeros_dev():
        return [jax.device_put(
            np.zeros((N_CORES * z.shape[0], *z.shape[1:]), z.dtype), sh)
            for z in zero_outs]
    out_arrs = sharded(*concat_in, *zeros_dev())
    jax.block_until_ready(out_arrs)

    exec_ns = None
    if time_iters:
        times = []
        for _ in range(time_iters):
            zs = zeros_dev()
            jax.block_until_ready(zs)
            t0 = _time.perf_counter()
            o = sharded(*concat_in, *zs)
            jax.block_until_ready(o)
            times.append(_time.perf_counter() - t0)
        exec_ns = int(min(times) * 1e9)

    results = [
        {name: np.asarray(out_arrs[i]).reshape(N_CORES, *out_avals[i].shape)[c]
         for i, name in enumerate(out_names)}
        for c in range(N_CORES)]
    return results, exec_ns


def kernel(x, w_q, w_k, w_v, w_o, gamma, beta):
    in_maps = _prep_inputs(x, w_q, w_k, w_v, w_o, gamma, beta)
    results, _ = _run_fast(in_maps)
    return _gather(results)


def kernel_profiled(x, w_q, w_k, w_v, w_o, gamma, beta, iters=10):
    """Returns (output, exec_time_ns or None)."""
    in_maps = _prep_inputs(x, w_q, w_k, w_v, w_o, gamma, beta)
    results, exec_ns = _run_fast(in_maps, time_iters=iters)
    return _gather(results), exec_ns
